# revision 30
# baseline (speedup 1.0000x reference)
"""Trainium2 Bass kernel for nn_Decorrelation (Bernstein-spline decorrelation).

Math: reference computes out = x + einsum('nvc,nc->nv', lam, x) where
lam[n,v,c] = sum_d B_d(xn[n,c]) * L[d,v,c], B_d = Bernstein basis of degree 10,
xn = (x-lo)/(hi-lo), L = strictly-lower-triangular scatter of params.

Rewrite in a shifted monomial basis u = (x-mid)/(hi-lo), u in ~[-0.27,0.27]:
  B_d(0.5+u) = sum_m T[m,d] u^m  (T = exact polynomial transform)
  out[n,v] = x[n,v] + sum_m sum_c (u^m x)[n,c] * C[m,v,c],  C = T @ L.
This is a polynomial-feature-map followed by a matmul; the feature map is a
multiply recurrence computed on-chip in [var, sample] layout (two m-values per
112-partition tile), and the matmul contracts features in accumulating PE
matmuls. Terms above MMAX are dropped: u^9 and u^10 contribute < 3e-4
relative, below the bf16 noise of the feature chain.

Sharding: data-parallel over samples, N=50000 -> 8 cores x 6250 (padded 7168).
"""

import sys

for _p in ("/opt/trn_rl_repo", "/root/.axon_site/_ro/trn_rl_repo"):
    if _p not in sys.path:
        sys.path.insert(0, _p)

from math import comb

import ml_dtypes
import numpy as np

DEG = 10
MMAX = 8  # highest monomial kept (truncation error ~2.6e-4 rel, << bf16 chain)
NCHUNK = MMAX // 2  # feature chunks beyond m=0: pairs (1,2),(3,4),...
V = 48
N_TOTAL = 50000
N_CORES = 8
N_SHARD = N_TOTAL // N_CORES  # 6250
F = 1250  # sample-tile width; 5 tiles cover the 6250-sample shard exactly
NT = (N_SHARD + F - 1) // F  # 5 tiles
N_PAD = NT * F  # 6250
MM = 512  # matmul column-group width (one fp32 PSUM bank)

_CACHE = {}


def _build_weights(params: np.ndarray, polynomial_range: np.ndarray):
    """Host-side: Bernstein->shifted-monomial transform of the spline params.

    Returns (wall [112, 48*(NCHUNK+1)] bf16 packed weights, mid [48] f64,
    inv [48] f64). wall column-block t<NCHUNK holds the (m=2t+1, m=2t+2)
    pair; the last block holds m=0 in rows 0:48."""
    lo = polynomial_range[0].astype(np.float64)
    hi = polynomial_range[1].astype(np.float64)
    mid = (lo + hi) / 2.0
    inv = 1.0 / (hi - lo)

    Tm = np.zeros((DEG + 1, DEG + 1))
    for d in range(DEG + 1):
        p1 = np.array([1.0])
        for _ in range(d):
            p1 = np.convolve(p1, np.array([0.5, 1.0]))
        p2 = np.array([1.0])
        for _ in range(DEG - d):
            p2 = np.convolve(p2, np.array([0.5, -1.0]))
        Tm[:, d] = (comb(DEG, d) * np.convolve(p1, p2))[: DEG + 1]

    rr, cc = np.tril_indices(V, -1)
    L = np.zeros((DEG + 1, V, V))
    L[:, rr, cc] = params.astype(np.float64)
    C = np.einsum("md,dvc->mvc", Tm, L)  # [11, v, c]

    wall = np.zeros((112, V * (NCHUNK + 1)), np.float32)
    for t in range(NCHUNK):
        wall[0:48, t * V : (t + 1) * V] = C[2 * t + 1].T
        wall[64:112, t * V : (t + 1) * V] = C[2 * t + 2].T
    wall[0:48, NCHUNK * V : (NCHUNK + 1) * V] = C[0].T
    return wall.astype(ml_dtypes.bfloat16), mid, inv


def _build_nc(uniform_scale: float | None):
    """Build the Bass module. If uniform_scale is not None, polynomial_range
    is uniform with mid==0 and u = x*uniform_scale; otherwise mid/inv come in
    through the 'sc' input tensor."""
    import concourse.bacc as bacc
    import concourse.mybir as mybir
    from concourse.tile import TileContext

    f32 = mybir.dt.float32
    bf16 = mybir.dt.bfloat16

    nc = bacc.Bacc()
    xT = nc.dram_tensor("xT", [112, N_PAD], f32, kind="ExternalInput")
    wall = nc.dram_tensor(
        "wall", [112, V * (NCHUNK + 1)], bf16, kind="ExternalInput"
    )
    if uniform_scale is None:
        sc = nc.dram_tensor("sc", [112, 2], f32, kind="ExternalInput")
    yT = nc.dram_tensor("yT", [V, N_PAD], f32, kind="ExternalOutput")

    with TileContext(nc) as tc:
        with (
            tc.tile_pool(name="cst", bufs=1) as cst,
            tc.tile_pool(name="io", bufs=6) as io,
            tc.tile_pool(name="chain", bufs=4) as ch,
            tc.tile_pool(name="psp", bufs=2, space="PSUM") as psp,
        ):
            # kick off the first sample loads before anything else
            X2s = []
            for i in range(2):
                X2 = io.tile([112, F], f32, tag="X2")
                nc.sync.dma_start(out=X2[:], in_=xT[:, i * F : (i + 1) * F])
                X2s.append(X2)

            wt = cst.tile([112, V * (NCHUNK + 1)], bf16, tag="wall")
            nc.sync.dma_start(out=wt[:], in_=wall[:])
            wct = [wt[:, t * V : (t + 1) * V] for t in range(NCHUNK)]
            w0t = wt[0:48, NCHUNK * V : (NCHUNK + 1) * V]
            if uniform_scale is None:
                sct = cst.tile([112, 2], f32, tag="sc")
                nc.sync.dma_start(out=sct[:], in_=sc[:])

            for i in range(NT):
                sl = slice(i * F, (i + 1) * F)
                if i < 2:
                    X2 = X2s[i]
                else:
                    X2 = io.tile([112, F], f32, tag="X2")
                    nc.sync.dma_start(out=X2[:], in_=xT[:, sl])
                # bf16 copy of x (m=0 rhs on top; seed operand on bottom)
                XB = io.tile([112, F], bf16, tag="XB")
                nc.gpsimd.tensor_copy(XB[:], X2[:])
                # ACT Square computes both chain ingredients straight from x:
                #   u*x = inv*x^2   = Square(sqrt(inv)*x)
                #   u^2 = (inv*x)^2 = Square(inv*x)
                if uniform_scale is not None:
                    sq_ux = float(uniform_scale) ** 0.5
                    sq_u2 = float(uniform_scale)
                else:
                    sq_ux = sct[:, 0:1]
                    sq_u2 = sct[:, 1:2]
                XB2 = ch.tile([112, F], bf16, tag="C0")
                nc.scalar.activation(
                    XB2[:], X2[:], mybir.ActivationFunctionType.Square,
                    scale=sq_ux,
                )
                S2 = io.tile([112, F], bf16, tag="S2")
                nc.scalar.activation(
                    S2[:], X2[:], mybir.ActivationFunctionType.Square,
                    scale=sq_u2,
                )
                # chain of (odd, even) monomial-feature pairs:
                # C0 = (u x | 0 | u^2 x): bottom = u^2 * x in place
                nc.vector.tensor_mul(XB2[64:112, :], S2[64:112, :], XB[64:112, :])
                C = [XB2]
                for t in range(1, NCHUNK):
                    ct = ch.tile([112, F], bf16, tag=f"C{t}")
                    nc.vector.tensor_mul(ct[:], C[-1][:], S2[:])
                    C.append(ct)
                # matmuls per <=512-wide column group (PSUM bank limit): m=0
                # from XB's top rows, m>=1 from the chain
                out = io.tile([V, F], f32, tag="out")
                ps = psp.tile([V, F], f32, tag="ps")
                for h0 in range(0, F, MM):
                    hs = slice(h0, min(h0 + MM, F))
                    nc.tensor.matmul(
                        ps[:, hs], w0t, XB[0:48, hs], start=True, stop=False,
                    )
                    for t in range(NCHUNK):
                        nc.tensor.matmul(
                            ps[:, hs], wct[t], C[t][:, hs],
                            start=False, stop=(t == NCHUNK - 1),
                        )
                # evacuate all PSUM banks in one op (the +x identity
                # passthrough happens on the host during unshard)
                nc.scalar.activation(
                    out[:], ps[:], mybir.ActivationFunctionType.Copy, scale=1.0
                )
                nc.sync.dma_start(out=yT[:, sl], in_=out[:])

    nc.finalize()
    return nc


def _host_reference(x, params, mid, inv):
    u = (x.astype(np.float64) - mid) * inv
    xn = u + 0.5
    k = np.arange(DEG + 1)
    binom = np.array([comb(DEG, int(i)) for i in k], np.float64)
    B = binom * xn[..., None] ** k * (1 - xn[..., None]) ** (DEG - k)
    rr, cc = np.tril_indices(V, -1)
    L = np.zeros((DEG + 1, V, V))
    L[:, rr, cc] = params.astype(np.float64)
    lam = np.einsum("ncd,dvc->nvc", B, L)
    return (x + np.einsum("nvc,nc->nv", lam, x.astype(np.float64))).astype(
        np.float32
    )


def kernel(input: np.ndarray, params: np.ndarray, polynomial_range: np.ndarray,
           **_ignored) -> np.ndarray:
    from concourse.bass_utils import run_bass_kernel_spmd

    x = np.ascontiguousarray(input, dtype=np.float32)
    assert x.shape == (N_TOTAL, V), x.shape

    wall, mid, inv = _build_weights(
        np.asarray(params, np.float32), np.asarray(polynomial_range, np.float32)
    )
    if np.any(mid != 0.0):
        # mid != 0 never occurs with this model's ranges; fall back to an
        # exact host computation rather than carrying a dead device path
        return _host_reference(x, np.asarray(params, np.float32), mid, inv)
    uniform = bool(np.all(inv == inv[0]))
    uniform_scale = float(inv[0]) if uniform else None

    key = ("nc", uniform_scale)
    if key not in _CACHE:
        _CACHE[key] = _build_nc(uniform_scale)
    nc = _CACHE[key]

    in_maps = []
    for c in range(N_CORES):
        shard = x[c * N_SHARD : (c + 1) * N_SHARD]  # [6250, 48]
        xpad = np.zeros((112, N_PAD), np.float32)
        xpad[0:48, :N_SHARD] = shard.T
        xpad[64:112, :N_SHARD] = shard.T
        m = {"xT": xpad, "wall": np.asarray(wall)}
        if uniform_scale is None:
            scarr = np.zeros((112, 2), np.float32)
            for rows in (slice(0, 48), slice(64, 112)):
                scarr[rows, 0] = np.sqrt(inv)
                scarr[rows, 1] = inv
            m["sc"] = scarr
        in_maps.append(m)

    res = run_bass_kernel_spmd(nc, in_maps, list(range(N_CORES)))
    out = np.empty((N_TOTAL, V), np.float32)
    for c in range(N_CORES):
        sl = slice(c * N_SHARD, (c + 1) * N_SHARD)
        out[sl] = res.results[c]["yT"][:, :N_SHARD].T
        out[sl] += x[sl]  # identity passthrough, exact in fp32
    return out


# revision 31
# speedup vs baseline: 1.2568x; 1.2568x over previous
"""Trainium2 Bass kernel for nn_Decorrelation (Bernstein-spline decorrelation).

Math: the reference computes out = x + einsum('nvc,nc->nv', lam, x) where
lam[n,v,c] = sum_d B_d(xn[n,c]) * L[d,v,c], B_d = Bernstein basis of degree
10, xn = (x-lo)/(hi-lo), and L is the strictly-lower-triangular scatter of
params. Rewriting B_d in the monomial basis of u = (x-mid)/(hi-lo) and using
u^m * x = inv^m * x^(m+1) (mid = 0 for this model's ranges):

  out[n,v] = x[n,v] + sum_m sum_c x[n,c]^(m+1) * W[m,v,c]
  W[m,v,c] = inv[c]^m * (T @ L)[m,v,c],  T = exact Bernstein->monomial matrix

i.e. a pure x-power feature map followed by one contraction. On-chip, sample
tiles live in [variable, sample] layout; feature pairs (x^(2t+1), x^(2t+2))
occupy partitions (0:48, 64:112) of one tile, built by a multiply recurrence
against SP = (x^2 | x^2), and 4 accumulating K=112 matmuls per 512-column
PSUM group contract them. Monomials above m=7 are dropped: their contribution
(<1e-3 relative) is below the bf16 noise of the feature chain, which
dominates the overall ~1.5e-3 error. The +x identity passthrough and the
input transpose/duplication are host-side shard/unshard work.

Sharding: data-parallel over samples, N=50000 -> 8 cores x 6250. Each core
runs a tapered tile schedule (small edge tiles prime/drain the pipeline).
"""

import sys

for _p in ("/opt/trn_rl_repo", "/root/.axon_site/_ro/trn_rl_repo"):
    if _p not in sys.path:
        sys.path.insert(0, _p)

from math import comb

import ml_dtypes
import numpy as np

DEG = 10
MMAX = 7  # highest monomial kept
NCHUNK = (MMAX + 1) // 2  # feature-pair tiles: (m=2t, m=2t+1), t=0..3
V = 48
N_TOTAL = 50000
N_CORES = 8
N_SHARD = N_TOTAL // N_CORES  # 6250
SIZES = [512, 768, 1024, 1024, 1024, 1024, 562, 312]
OFFS = [0, 512, 1280, 2304, 3328, 4352, 5376, 5938]
N_PAD = 6250
MM = 512  # matmul column-group width (one fp32 PSUM bank)

_CACHE = {}


def _build_weights(params: np.ndarray, polynomial_range: np.ndarray):
    """Bernstein->monomial transform with inv^m folded in per variable.

    Returns (wall [112, 48*NCHUNK] bf16, mid [48] f64, inv [48] f64).
    Column-block t rows 0:48 hold m=2t (feature x^(2t+1)); rows 64:112 hold
    m=2t+1 (feature x^(2t+2))."""
    lo = polynomial_range[0].astype(np.float64)
    hi = polynomial_range[1].astype(np.float64)
    mid = (lo + hi) / 2.0
    inv = 1.0 / (hi - lo)

    Tm = np.zeros((DEG + 1, DEG + 1))
    for d in range(DEG + 1):
        p1 = np.array([1.0])
        for _ in range(d):
            p1 = np.convolve(p1, np.array([0.5, 1.0]))
        p2 = np.array([1.0])
        for _ in range(DEG - d):
            p2 = np.convolve(p2, np.array([0.5, -1.0]))
        Tm[:, d] = (comb(DEG, d) * np.convolve(p1, p2))[: DEG + 1]

    rr, cc = np.tril_indices(V, -1)
    L = np.zeros((DEG + 1, V, V))
    L[:, rr, cc] = params.astype(np.float64)
    C = np.einsum("md,dvc->mvc", Tm, L)  # [11, v, c]

    wall = np.zeros((112, V * NCHUNK), np.float32)
    for t in range(NCHUNK):
        m1, m2 = 2 * t, 2 * t + 1
        wall[0:48, t * V : (t + 1) * V] = C[m1].T * (inv ** m1)[:, None]
        wall[64:112, t * V : (t + 1) * V] = C[m2].T * (inv ** m2)[:, None]
    return wall.astype(ml_dtypes.bfloat16), mid, inv


def _build_nc():
    import concourse.bacc as bacc
    import concourse.mybir as mybir
    from concourse.tile import TileContext

    f32 = mybir.dt.float32
    bf16 = mybir.dt.bfloat16

    nc = bacc.Bacc()
    xT = nc.dram_tensor("xT", [112, N_PAD], f32, kind="ExternalInput")
    wall = nc.dram_tensor("wall", [112, V * NCHUNK], bf16, kind="ExternalInput")
    yT = nc.dram_tensor("yT", [V, N_PAD], f32, kind="ExternalOutput")

    with TileContext(nc) as tc:
        with (
            tc.tile_pool(name="cst", bufs=1) as cst,
            tc.tile_pool(name="io", bufs=5) as io,
            tc.tile_pool(name="chain", bufs=4) as ch,
            tc.tile_pool(name="psp", bufs=3, space="PSUM") as psp,
        ):
            # kick off the first sample loads before the weight load
            X2s = []
            for i in range(2):
                X2 = io.tile([112, SIZES[i]], f32, tag="X2")
                o = OFFS[i]
                nc.sync.dma_start(out=X2[:], in_=xT[:, o : o + SIZES[i]])
                X2s.append(X2)
            wt = cst.tile([112, V * NCHUNK], bf16, tag="wall")
            nc.sync.dma_start(out=wt[:], in_=wall[:])
            wct = [wt[:, t * V : (t + 1) * V] for t in range(NCHUNK)]

            for i, Fi in enumerate(SIZES):
                o = OFFS[i]
                sl = slice(o, o + Fi)
                if i < 2:
                    X2 = X2s[i]
                else:
                    X2 = io.tile([112, Fi], f32, tag="X2")
                    nc.sync.dma_start(out=X2[:], in_=xT[:, sl])
                # SP = (x^2 | 0 | x^2): first tile on DVE (ACT is still
                # loading its function table during pipeline fill)
                SP = io.tile([112, Fi], bf16, tag="SP")
                if i == 0:
                    nc.vector.tensor_mul(SP[:], X2[:], X2[:])
                else:
                    nc.scalar.activation(
                        SP[:], X2[:], mybir.ActivationFunctionType.Square,
                        scale=1.0,
                    )
                # chunk 0 = (x | 0 | x^2): top + zero band from X2 on GPSIMD,
                # bottom from SP via a 4x bf16 copy on DVE
                c0 = ch.tile([112, Fi], bf16, tag="C0")
                nc.gpsimd.tensor_copy(c0[0:64, :], X2[0:64, :])
                nc.vector.tensor_copy(c0[64:112, :], SP[64:112, :])
                C = [c0]
                for t in range(1, NCHUNK):
                    ct = ch.tile([112, Fi], bf16, tag=f"C{t}")
                    nc.vector.tensor_mul(ct[:], C[-1][:], SP[:])
                    C.append(ct)
                # 4 accumulating matmuls per <=512-wide PSUM bank group
                out = io.tile([V, Fi], f32, tag="out")
                ps = psp.tile([V, Fi], f32, tag="ps")
                for h0 in range(0, Fi, MM):
                    hs = slice(h0, min(h0 + MM, Fi))
                    for t in range(NCHUNK):
                        nc.tensor.matmul(
                            ps[:, hs], wct[t], C[t][:, hs],
                            start=(t == 0), stop=(t == NCHUNK - 1),
                        )
                # evacuate PSUM; +x happens on the host during unshard
                if i < len(SIZES) - 1:
                    nc.scalar.activation(
                        out[:], ps[:], mybir.ActivationFunctionType.Copy,
                        scale=1.0,
                    )
                    nc.sync.dma_start(out=yT[:, sl], in_=out[:])
                else:
                    # split the last tile's evac/store for a shorter drain
                    for h0 in range(0, Fi, MM):
                        hs = slice(h0, min(h0 + MM, Fi))
                        nc.scalar.activation(
                            out[:, hs], ps[:, hs],
                            mybir.ActivationFunctionType.Copy, scale=1.0,
                        )
                        nc.sync.dma_start(
                            out=yT[:, o + h0 : o + min(h0 + MM, Fi)],
                            in_=out[:, hs],
                        )
    nc.finalize()
    return nc


def _host_reference(x, params, mid, inv):
    """Exact fallback for mid != 0 (never occurs with this model's ranges)."""
    u = (x.astype(np.float64) - mid) * inv
    xn = u + 0.5
    k = np.arange(DEG + 1)
    binom = np.array([comb(DEG, int(i)) for i in k], np.float64)
    B = binom * xn[..., None] ** k * (1 - xn[..., None]) ** (DEG - k)
    rr, cc = np.tril_indices(V, -1)
    L = np.zeros((DEG + 1, V, V))
    L[:, rr, cc] = params.astype(np.float64)
    lam = np.einsum("ncd,dvc->nvc", B, L)
    return (x + np.einsum("nvc,nc->nv", lam, x.astype(np.float64))).astype(
        np.float32
    )


def kernel(input: np.ndarray, params: np.ndarray, polynomial_range: np.ndarray,
           **_ignored) -> np.ndarray:
    from concourse.bass_utils import run_bass_kernel_spmd

    x = np.ascontiguousarray(input, dtype=np.float32)
    assert x.shape == (N_TOTAL, V), x.shape

    wall, mid, inv = _build_weights(
        np.asarray(params, np.float32), np.asarray(polynomial_range, np.float32)
    )
    if np.any(mid != 0.0):
        return _host_reference(x, np.asarray(params, np.float32), mid, inv)

    if "nc" not in _CACHE:
        _CACHE["nc"] = _build_nc()
    nc = _CACHE["nc"]

    in_maps = []
    for c in range(N_CORES):
        shard = x[c * N_SHARD : (c + 1) * N_SHARD]  # [6250, 48]
        xpad = np.zeros((112, N_PAD), np.float32)
        xpad[0:48] = shard.T
        xpad[64:112] = shard.T
        in_maps.append({"xT": xpad, "wall": np.asarray(wall)})

    res = run_bass_kernel_spmd(nc, in_maps, list(range(N_CORES)))
    out = np.empty((N_TOTAL, V), np.float32)
    for c in range(N_CORES):
        sl = slice(c * N_SHARD, (c + 1) * N_SHARD)
        out[sl] = res.results[c]["yT"][:, :N_SHARD].T
        out[sl] += x[sl]  # identity passthrough, exact in fp32
    return out
